# revision 1
# baseline (speedup 1.0000x reference)
"""ClusterAttention (segment_reduce) Trainium2 kernel.

Strategy: shard by cluster ("bucket"). The host groups point indices by
bucket (argsort of cluster_ids — pure index preprocessing), pads each
bucket's point list to a multiple of 16 (a "slot") by duplicating the
bucket's first point, and cuts the bucket list into 8 contiguous,
point-balanced core shards. Every core therefore owns whole buckets and
needs NO cross-core communication:

  pass 1: x -> transpose -> k^T,v^T matmuls; per-slot sums of x (PE one-hot
          matmuls, pad-masked) and per-slot maxes of k^T,v^T (DVE reduce).
  mid:    slot->bucket combine (gpsimd ap_gather + DVE reduce), then build
          per-bucket context tables using the linearity of segment-sum
          (k_sum[b] = x_sum[b] @ Wk + cnt*bk):
            tbl[...,0] = scale*(x_mean@Wk + bk)        (k_mean, pre-scaled)
            tbl[...,1] = scale*(max_k + bk)            (k_max,  pre-scaled)
            tbl[...,2] = [v_mean|v_max] @ Wvc + bvc    (v_combined)
          and expand per-slot context (ap_gather).
  pass 2: x^T (host-pretransposed) -> q^T; interaction = (q^T+bq)*ctx
          (slot-broadcast); gate MLP; out^T = Wp.T @ (gate*v_comb) + bp.

The output is returned transposed+permuted; the host scatters rows back
(duplicate pad rows rewrite identical values).
"""

import numpy as np

import concourse.bass as bass
from concourse import bacc
import concourse.mybir as mybir
import concourse.tile as tile
import concourse.bass_utils as bass_utils
from concourse.masks import make_identity

# problem constants (hardcoded per contract)
N_GLOBAL = 500000
C = 128
B_GLOBAL = 8192
NUM_HEADS = 4
NCORES = 8
SLOT = 16          # points per slot (pad unit)
CHUNK = 512        # points per device chunk
TPC = CHUNK // 128  # 128-row tiles per chunk
SCALE = float((C // NUM_HEADS) ** (-0.5))
NEG_BIG = -1.0e30

f32 = mybir.dt.float32
f32r = mybir.dt.float32r
i16 = mybir.dt.int16
X = mybir.AxisListType.X
ALU = mybir.AluOpType
ACTF = mybir.ActivationFunctionType


def _wrap16(vals):
    """ap_gather index layout: [128, n//16] int16, idx j read from
    partition j%16 (replicated across the 8 gpsimd cores' groups)."""
    v = np.asarray(vals, np.int16)
    n = v.size
    assert n % 16 == 0
    arr = np.zeros((128, n // 16), np.int16)
    k = np.arange(n)
    for g in range(8):
        arr[16 * g + (k % 16), k // 16] = v
    return arr


def _build_layout(ids, B, ncores):
    """Bucket-grouped, slot-padded permutation + all per-core metadata."""
    N = ids.shape[0]
    counts = np.bincount(ids, minlength=B).astype(np.int64)
    order = np.argsort(ids, kind="stable")
    starts = np.zeros(B + 1, np.int64)
    starts[1:] = np.cumsum(counts)
    nslots_b = (counts + SLOT - 1) // SLOT
    padded_b = nslots_b * SLOT

    cum = np.cumsum(padded_b)
    total = cum[-1]
    cuts = [0]
    for c in range(1, ncores):
        cuts.append(int(np.searchsorted(cum, c * total / ncores)))
    cuts.append(B)

    core_npts = [int(padded_b[cuts[c]:cuts[c + 1]].sum()) for c in range(ncores)]
    core_nbux = [cuts[c + 1] - cuts[c] for c in range(ncores)]
    NPTS = max(2 * CHUNK, int(-(-max(core_npts) // (2 * CHUNK))) * 2 * CHUNK)
    NBUX = max(16, int(-(-max(core_nbux) // 16)) * 16)
    NSLOT = NPTS // SLOT
    SPB = max(1, int(nslots_b.max()))  # max slots per bucket
    NTILE = NPTS // 128

    cores = []
    for c in range(ncores):
        bs, be = cuts[c], cuts[c + 1]
        nb = be - bs
        cnts = counts[bs:be]
        nsl = nslots_b[bs:be]
        sstart = np.zeros(nb + 1, np.int64)
        sstart[1:] = np.cumsum(nsl)
        tot_slots = int(sstart[-1])
        dst0 = sstart[:-1] * SLOT

        perm = np.full(NPTS, -1, np.int64)
        is_real = np.zeros(NPTS, bool)
        src = order[starts[bs]:starts[be]]
        if src.size:
            seg0 = (starts[bs:be] - starts[bs])
            pos = np.repeat(dst0, cnts) + (np.arange(src.size) - np.repeat(seg0, cnts))
            perm[pos] = src
            is_real[pos] = True
        padcnt = (nsl * SLOT - cnts)
        if padcnt.sum():
            off = np.concatenate([[0], np.cumsum(padcnt)])[:-1]
            ppos = np.repeat(dst0 + cnts, padcnt) + (np.arange(int(padcnt.sum())) - np.repeat(off, padcnt))
            first = np.where(cnts > 0, order[starts[bs:be]], 0)
            perm[ppos] = np.repeat(first, padcnt)

        si = np.zeros((NPTS, 8), np.float32)
        p_idx = np.arange(NPTS) % 128
        si[np.arange(NPTS), p_idx // 16] = is_real.astype(np.float32)
        slot_ind = si.reshape(NTILE, 128, 8)

        cnt_rep = np.ones(NBUX, np.float32)
        cnt_rep[:nb] = np.maximum(cnts, 1).astype(np.float32)
        cnt_rep = np.broadcast_to(cnt_rep, (128, NBUX)).copy()

        jj = np.arange(SPB)[None, :]
        bmat = np.where(jj < nsl[:, None], sstart[:-1][:, None] + jj, NSLOT)
        L = np.full(NBUX * SPB, NSLOT, np.int64)
        L[: nb * SPB] = bmat.ravel()
        bidx = _wrap16(L)

        S = np.full(NSLOT, NBUX, np.int64)
        if tot_slots:
            S[:tot_slots] = np.repeat(np.arange(nb), nsl)
        sidx = _wrap16(S)

        cores.append(dict(perm=perm, slot_ind=slot_ind, cnt=cnt_rep,
                          bidx=bidx, sidx=sidx))
    return dict(NPTS=NPTS, NBUX=NBUX, NSLOT=NSLOT, SPB=SPB, NTILE=NTILE,
                NCHUNK=NPTS // CHUNK, cores=cores)


def _build_nc(L, reps=1):
    NPTS, NBUX, NSLOT, SPB = L["NPTS"], L["NBUX"], L["NSLOT"], L["SPB"]
    NTILE, NCHUNK = L["NTILE"], L["NCHUNK"]

    nc = bacc.Bacc("TRN2", target_bir_lowering=False)
    xp = nc.dram_tensor("xp", [NPTS, C], f32, kind="ExternalInput")
    xtp = nc.dram_tensor("xtp", [C, NPTS], f32r, kind="ExternalInput")
    sind_d = nc.dram_tensor("sind", [NTILE, 128, 8], f32, kind="ExternalInput")
    cnt_d = nc.dram_tensor("cnt", [128, NBUX], f32, kind="ExternalInput")
    bidx_d = nc.dram_tensor("bidx", [128, (NBUX * SPB) // 16], i16, kind="ExternalInput")
    sidx_d = nc.dram_tensor("sidx", [128, NSLOT // 16], i16, kind="ExternalInput")
    w_d = {}
    for nm, sh in [("Wq", [C, C]), ("Wk", [C, C]), ("Wv", [C, C]),
                   ("Wg1", [2 * C, C]), ("Wg2", [C, C]), ("Wvc", [2 * C, C]),
                   ("Wp", [C, C])]:
        w_d[nm] = nc.dram_tensor(nm, sh, f32, kind="ExternalInput")
    b_d = {}
    for nm in ["bq", "bk", "bv", "bg1", "bg2", "bvc", "bp"]:
        b_d[nm] = nc.dram_tensor(nm, [C], f32, kind="ExternalInput")
    ot = nc.dram_tensor("ot", [C, NPTS], f32, kind="ExternalOutput")

    xv2 = xp[:].rearrange("(c t p) f -> c p t f", p=128, t=2 * TPC)

    def _emit(tc):
        with tc.tile_pool(name="consts", bufs=1) as consts, \
             tc.tile_pool(name="tables", bufs=1) as tables:
            ident = consts.tile([128, 128], f32)
            make_identity(nc, ident[:])
            w = {}
            for nm in ["Wq", "Wk", "Wv", "Wg2", "Wp"]:
                w[nm] = consts.tile([C, C], f32r, name="w_" + nm, tag="w_" + nm)
                nc.sync.dma_start(out=w[nm][:], in_=w_d[nm][:].bitcast(f32r))
            for nm, sl in [("Wg1a", 0), ("Wg1b", 1)]:
                w[nm] = consts.tile([C, C], f32r, name="w_" + nm, tag="w_" + nm)
                nc.sync.dma_start(out=w[nm][:],
                                  in_=w_d["Wg1"][sl * C:(sl + 1) * C, :].bitcast(f32r))
            for nm, sl in [("Wvca", 0), ("Wvcb", 1)]:
                w[nm] = consts.tile([C, C], f32, name="w_" + nm, tag="w_" + nm)
                nc.sync.dma_start(out=w[nm][:], in_=w_d["Wvc"][sl * C:(sl + 1) * C, :])
            w32 = {}
            for nm in ["Wk", "Wv"]:
                w32[nm] = consts.tile([C, C], f32, name="w32_" + nm, tag="w32_" + nm)
                nc.sync.dma_start(out=w32[nm][:], in_=w_d[nm][:])
            b = {}
            for nm in b_d:
                b[nm] = consts.tile([C, 1], f32, name="b_" + nm, tag="b_" + nm)
                nc.sync.dma_start(out=b[nm][:], in_=b_d[nm][:, None])
            bk_s = consts.tile([C, 1], f32)
            nc.scalar.activation(out=bk_s[:], in_=b["bk"][:], func=ACTF.Identity,
                                 scale=SCALE)

            # ---------------- pass 1 ----------------
            with tc.tile_pool(name="slots", bufs=1) as slots, \
                 tc.tile_pool(name="ctx", bufs=1) as ctxp:
                xslot = slots.tile([128, NSLOT + 1], f32)
                kvslot = slots.tile([128, 2, NSLOT + 1], f32)
                nc.vector.memset(xslot[:, NSLOT:], 0.0)
                nc.vector.memset(kvslot[:, :, NSLOT:], NEG_BIG)

                with tc.tile_pool(name="p1w", bufs=3) as p1w, \
                     tc.tile_pool(name="p1ps", bufs=2, space="PSUM") as p1ps, \
                     tc.tile_pool(name="p1xs", bufs=2, space="PSUM") as p1xs:
                    sind_sb = p1w.tile([128, NTILE, 8], f32, bufs=1)
                    nc.sync.dma_start(out=sind_sb[:],
                                      in_=sind_d[:].rearrange("a p s -> p a s"))
                    for cj in range(NCHUNK // 2):
                        xt8 = p1w.tile([128, 2 * TPC, 128], f32, tag="xt8")
                        nc.sync.dma_start(out=xt8[:], in_=xv2[cj])
                        for half in range(2):
                            ci = cj * 2 + half
                            xT_ps = p1ps.tile([128, CHUNK], f32, tag="xT")
                            for t in range(TPC):
                                nc.tensor.transpose(
                                    out=xT_ps[:, t * 128:(t + 1) * 128],
                                    in_=xt8[:, half * TPC + t, :], identity=ident[:])
                            xT_sb = p1w.tile([128, CHUNK], f32r, tag="xTs")
                            nc.scalar.copy(out=xT_sb[:], in_=xT_ps[:])
                            kv_ps = p1ps.tile([128, 2, CHUNK], f32, tag="kv")
                            nc.tensor.matmul(out=kv_ps[:, 0, :], lhsT=w["Wk"][:],
                                             rhs=xT_sb[:], start=True, stop=True)
                            nc.tensor.matmul(out=kv_ps[:, 1, :], lhsT=w["Wv"][:],
                                             rhs=xT_sb[:], start=True, stop=True)
                            xs_ps = p1xs.tile([128, 32], f32, tag="xs")
                            for t in range(TPC):
                                nc.tensor.matmul(
                                    out=xs_ps[:, t * 8:(t + 1) * 8],
                                    lhsT=xt8[:, half * TPC + t, :],
                                    rhs=sind_sb[:, ci * TPC + t, :],
                                    start=True, stop=True)
                            nc.scalar.copy(out=xslot[:, ci * 32:ci * 32 + 32],
                                           in_=xs_ps[:])
                            nc.vector.tensor_reduce(
                                out=kvslot[:, :, ci * 32:ci * 32 + 32],
                                in_=kv_ps[:].rearrange("p u (s e) -> p (u s) e",
                                                       e=SLOT),
                                axis=X, op=ALU.max)

                # ---------------- mid phase ----------------
                tbl = ctxp.tile([128, NBUX + 1, 3], f32)
                ctxslot = tables.tile([128, NSLOT, 3], f32)
                with tc.tile_pool(name="midw", bufs=1) as midw, \
                     tc.tile_pool(name="midps", bufs=2, space="PSUM") as midps:
                    bidx_sb = midw.tile([128, (NBUX * SPB) // 16], i16, tag="bidx")
                    nc.sync.dma_start(out=bidx_sb[:], in_=bidx_d[:])
                    sidx_sb = midw.tile([128, NSLOT // 16], i16, tag="sidx")
                    nc.sync.dma_start(out=sidx_sb[:], in_=sidx_d[:])
                    cnt_sb = midw.tile([128, NBUX], f32, tag="cnt")
                    nc.sync.dma_start(out=cnt_sb[:], in_=cnt_d[:])
                    rc = midw.tile([128, NBUX], f32, tag="rc")
                    nc.vector.reciprocal(out=rc[:], in_=cnt_sb[:])

                    BKB = 512  # buckets per gather block
                    red = {}
                    for nm, src_t, op in [("xbsum", xslot[:, :, None], ALU.add),
                                          ("kbmax", kvslot[:, 0, :, None], ALU.max),
                                          ("vbmax", kvslot[:, 1, :, None], ALU.max)]:
                        red[nm] = midw.tile([128, NBUX], f32, name="red_" + nm, tag="red_" + nm)
                        for j in range(0, NBUX, BKB):
                            e = min(j + BKB, NBUX)
                            nbk = e - j
                            g = midw.tile([128, nbk * SPB], f32, tag="gst", bufs=2)
                            nc.gpsimd.ap_gather(
                                out_ap=g[:], in_ap=src_t,
                                idxs_ap=bidx_sb[:, (j * SPB) // 16:(e * SPB) // 16],
                                channels=128,
                                num_elems=NSLOT + 1, d=1, num_idxs=nbk * SPB)
                            nc.vector.tensor_reduce(
                                out=red[nm][:, j:e],
                                in_=g[:].rearrange("p (b j) -> p b j", j=SPB),
                                axis=X, op=op)

                    xmean = midw.tile([128, NBUX], f32, tag="xmean")
                    nc.vector.tensor_tensor(out=xmean[:], in0=red["xbsum"][:],
                                            in1=rc[:], op=ALU.mult)

                    def mm_big(ps, lhsT, rhs_t):
                        for j in range(0, NBUX, 512):
                            e = min(j + 512, NBUX)
                            nc.tensor.matmul(out=ps[:, j:e], lhsT=lhsT,
                                             rhs=rhs_t[:, j:e], start=True, stop=True)

                    km_ps = midps.tile([128, NBUX], f32, tag="mmp")
                    mm_big(km_ps, w32["Wk"][:], xmean)
                    nc.scalar.activation(out=tbl[:, :NBUX, 0], in_=km_ps[:],
                                         func=ACTF.Identity, scale=SCALE, bias=bk_s[:])
                    nc.scalar.activation(out=tbl[:, :NBUX, 1], in_=red["kbmax"][:],
                                         func=ACTF.Identity, scale=SCALE, bias=bk_s[:])

                    vm_ps = midps.tile([128, NBUX], f32, tag="mmp")
                    mm_big(vm_ps, w32["Wv"][:], xmean)
                    vmean = midw.tile([128, NBUX], f32, tag="vmean")
                    nc.scalar.activation(out=vmean[:], in_=vm_ps[:],
                                         func=ACTF.Identity, bias=b["bv"][:])
                    vmax = midw.tile([128, NBUX], f32, tag="vmax")
                    nc.scalar.activation(out=vmax[:], in_=red["vbmax"][:],
                                         func=ACTF.Identity, bias=b["bv"][:])
                    vc_ps = midps.tile([128, NBUX], f32, tag="mmp")
                    for j in range(0, NBUX, 512):
                        e = min(j + 512, NBUX)
                        nc.tensor.matmul(out=vc_ps[:, j:e], lhsT=w["Wvca"][:],
                                         rhs=vmean[:, j:e], start=True, stop=False)
                        nc.tensor.matmul(out=vc_ps[:, j:e], lhsT=w["Wvcb"][:],
                                         rhs=vmax[:, j:e], start=False, stop=True)
                    nc.scalar.activation(out=tbl[:, :NBUX, 2], in_=vc_ps[:],
                                         func=ACTF.Identity, bias=b["bvc"][:])
                    nc.vector.memset(tbl[:, NBUX, :], 0.0)

                    CTXB = max(16, (-(-NSLOT // 8) // 16) * 16)
                    for j in range(0, NSLOT, CTXB):
                        e = min(j + CTXB, NSLOT)
                        nc.gpsimd.ap_gather(
                            out_ap=ctxslot[:, j:e, :], in_ap=tbl[:],
                            idxs_ap=sidx_sb[:, j // 16:e // 16], channels=128,
                            num_elems=NBUX + 1, d=3, num_idxs=e - j)

            # ---------------- pass 2 ----------------
            with tc.tile_pool(name="p2w", bufs=3) as p2w, \
                 tc.tile_pool(name="p2ps", bufs=2, space="PSUM") as p2ps, \
                 tc.tile_pool(name="p2po", bufs=2, space="PSUM") as p2po:
                for cj in range(NCHUNK // 2):
                    xT2 = p2w.tile([128, 2 * CHUNK], f32r, tag="xT2")
                    nc.sync.dma_start(out=xT2[:],
                                      in_=xtp[:, cj * 2 * CHUNK:(cj + 1) * 2 * CHUNK])
                    oT2 = p2w.tile([128, 2, CHUNK], f32, tag="oTs")
                    for half in range(2):
                        ci = cj * 2 + half
                        sl0 = ci * 32
                        qT_ps = p2ps.tile([128, CHUNK], f32, tag="qT")
                        nc.tensor.matmul(
                            out=qT_ps[:], lhsT=w["Wq"][:],
                            rhs=xT2[:, half * CHUNK:(half + 1) * CHUNK],
                            start=True, stop=True)
                        ctx = ctxslot[:, sl0:sl0 + 32, :]
                        inter = p2w.tile([128, 2, 32, SLOT], f32r, tag="inter")
                        qv = qT_ps[:].rearrange("p (s e) -> p s e", e=SLOT)
                        for u in range(2):
                            nc.vector.scalar_tensor_tensor(
                                out=inter[:, u], in0=qv, scalar=b["bq"][:],
                                in1=ctx[:, :, u:u + 1].broadcast_to([128, 32, SLOT]),
                                op0=ALU.add, op1=ALU.mult)
                        h1_ps = p2ps.tile([128, CHUNK], f32, tag="h1")
                        nc.tensor.matmul(
                            out=h1_ps[:], lhsT=w["Wg1a"][:],
                            rhs=inter[:, 0].rearrange("p a b -> p (a b)"),
                            start=True, stop=False)
                        nc.tensor.matmul(
                            out=h1_ps[:], lhsT=w["Wg1b"][:],
                            rhs=inter[:, 1].rearrange("p a b -> p (a b)"),
                            start=False, stop=True)
                        h1 = p2w.tile([128, CHUNK], f32r, tag="h1s")
                        nc.scalar.activation(out=h1[:], in_=h1_ps[:], func=ACTF.Relu,
                                             bias=b["bg1"][:])
                        h2_ps = p2ps.tile([128, CHUNK], f32, tag="h2")
                        nc.tensor.matmul(out=h2_ps[:], lhsT=w["Wg2"][:],
                                         rhs=h1[:], start=True, stop=True)
                        gate = p2w.tile([128, CHUNK], f32, tag="gate")
                        nc.scalar.activation(out=gate[:], in_=h2_ps[:],
                                             func=ACTF.Sigmoid, bias=b["bg2"][:])
                        gv = p2w.tile([128, 32, SLOT], f32r, tag="gv")
                        nc.gpsimd.tensor_tensor(
                            out=gv[:],
                            in0=gate[:].rearrange("p (s e) -> p s e", e=SLOT),
                            in1=ctx[:, :, 2:3].broadcast_to([128, 32, SLOT]),
                            op=ALU.mult)
                        oT_ps = p2po.tile([128, CHUNK], f32, tag="oT")
                        nc.tensor.matmul(
                            out=oT_ps[:], lhsT=w["Wp"][:],
                            rhs=gv[:].rearrange("p a b -> p (a b)"),
                            start=True, stop=True)
                        if ci % 2 == 0:
                            nc.vector.tensor_scalar(out=oT2[:, half, :], in0=oT_ps[:],
                                                    scalar1=b["bp"][:], scalar2=None,
                                                    op0=ALU.add)
                        else:
                            nc.scalar.activation(out=oT2[:, half, :], in_=oT_ps[:],
                                                 func=ACTF.Identity, bias=b["bp"][:])
                    nc.gpsimd.dma_start(
                        out=ot[:, cj * 2 * CHUNK:(cj + 1) * 2 * CHUNK], in_=oT2[:])

    with tile.TileContext(nc) as tc:
        for _rep in range(reps):
            _emit(tc)
    nc.finalize()
    return nc


def _make_in_maps(inputs, layout):
    shared = {nm: np.ascontiguousarray(inputs[nm], np.float32)
              for nm in ["Wq", "Wk", "Wv", "Wg1", "Wg2", "Wvc", "Wp",
                         "bq", "bk", "bv", "bg1", "bg2", "bvc", "bp"]}
    x = np.ascontiguousarray(inputs["x"], np.float32)
    in_maps = []
    for core in layout["cores"]:
        perm = core["perm"]
        xp = np.zeros((layout["NPTS"], C), np.float32)
        m = perm >= 0
        xp[m] = x[perm[m]]
        in_maps.append(dict(shared, xp=xp, xtp=np.ascontiguousarray(xp.T),
                            sind=core["slot_ind"], cnt=core["cnt"],
                            bidx=core["bidx"], sidx=core["sidx"]))
    return in_maps


def _assemble_out(results, layout, n):
    out = np.empty((n, C), np.float32)
    for core, r in zip(layout["cores"], results):
        perm = core["perm"]
        m = perm >= 0
        out[perm[m]] = r["ot"].T[m]
    return out


def _run(inputs, layout, trace=False):
    nc = _build_nc(layout)
    in_maps = _make_in_maps(inputs, layout)
    res = bass_utils.run_bass_kernel_spmd(
        nc, in_maps, core_ids=list(range(NCORES)), trace=trace)
    out = _assemble_out(res.results, layout, inputs["x"].shape[0])
    return out, res


def kernel(**inputs):
    ids = np.asarray(inputs["cluster_ids"]).astype(np.int64)
    B = int(inputs["total_buckets"])
    layout = _build_layout(ids, B, NCORES)
    out, _ = _run(inputs, layout, trace=False)
    return out


# ---------------------------------------------------------------------------
# pure-numpy emulation of the device program (for logic validation off-HW)
def kernel_emulate(**inputs):
    ids = np.asarray(inputs["cluster_ids"]).astype(np.int64)
    B = int(inputs["total_buckets"])
    L = _build_layout(ids, B, NCORES)
    NPTS, NBUX, NSLOT, SPB = L["NPTS"], L["NBUX"], L["NSLOT"], L["SPB"]
    x = np.asarray(inputs["x"], np.float32)
    W = {k: np.asarray(inputs[k], np.float32) for k in
         ["Wq", "Wk", "Wv", "Wg1", "Wg2", "Wvc", "Wp",
          "bq", "bk", "bv", "bg1", "bg2", "bvc", "bp"]}
    n = x.shape[0]
    out = np.empty((n, C), np.float32)
    for core in L["cores"]:
        perm = core["perm"]
        m = perm >= 0
        xp = np.zeros((NPTS, C), np.float32)
        xp[m] = x[perm[m]]
        sind = core["slot_ind"].reshape(NPTS, 8)
        # pass 1
        kT = (xp @ W["Wk"]).T  # pre-bias
        vT = (xp @ W["Wv"]).T
        xslot = np.zeros((128, NSLOT + 1), np.float32)
        kslot = np.full((128, NSLOT + 1), NEG_BIG, np.float32)
        vslot = np.full((128, NSLOT + 1), NEG_BIG, np.float32)
        # slot sums via indicator (pads zeroed), slot maxes direct
        ind = np.zeros((NPTS, NSLOT), np.float32)
        srow = np.arange(NPTS) // SLOT
        ind[np.arange(NPTS), srow] = sind[np.arange(NPTS), (np.arange(NPTS) % 128) // 16]
        xslot[:, :NSLOT] = xp.T @ ind
        kslot[:, :NSLOT] = kT.reshape(128, NSLOT, SLOT).max(axis=2)
        vslot[:, :NSLOT] = vT.reshape(128, NSLOT, SLOT).max(axis=2)
        # mid
        def unwrap(arr, n):
            outv = np.zeros(n, np.int64)
            k = np.arange(n)
            outv[k] = arr[(k % 16), k // 16]
            return outv
        bidx = unwrap(core["bidx"], NBUX * SPB)
        sidx = unwrap(core["sidx"], NSLOT)
        g = xslot[:, bidx].reshape(128, NBUX, SPB)
        xbsum = g.sum(axis=2)
        kbmax = kslot[:, bidx].reshape(128, NBUX, SPB).max(axis=2)
        vbmax = vslot[:, bidx].reshape(128, NBUX, SPB).max(axis=2)
        rc = 1.0 / core["cnt"]
        xmean = xbsum * rc
        tbl = np.zeros((128, NBUX + 1, 3), np.float32)
        tbl[:, :NBUX, 0] = SCALE * (W["Wk"].T @ xmean + W["bk"][:, None])
        tbl[:, :NBUX, 1] = SCALE * (kbmax + W["bk"][:, None])
        vmean = W["Wv"].T @ xmean + W["bv"][:, None]
        vmax = vbmax + W["bv"][:, None]
        tbl[:, :NBUX, 2] = (W["Wvc"][:C].T @ vmean + W["Wvc"][C:].T @ vmax
                            + W["bvc"][:, None])
        ctxslot = tbl[:, sidx, :]  # [128, NSLOT, 3]
        # pass 2
        qT = (xp @ W["Wq"]).T + W["bq"][:, None]
        ctxe = np.repeat(ctxslot, SLOT, axis=1)  # [128, NPTS, 3]
        inter1 = qT * ctxe[:, :, 0]
        inter2 = qT * ctxe[:, :, 1]
        h1 = np.maximum(W["Wg1"][:C].T @ inter1 + W["Wg1"][C:].T @ inter2
                        + W["bg1"][:, None], 0.0)
        h2 = W["Wg2"].T @ h1 + W["bg2"][:, None]
        gate = 1.0 / (1.0 + np.exp(-h2))
        gv = gate * ctxe[:, :, 2]
        oT = W["Wp"].T @ gv + W["bp"][:, None]
        out[perm[m]] = oT.T[m]
    return out



# revision 10
# speedup vs baseline: 10.7801x; 10.7801x over previous
"""ClusterAttention (segment_reduce) Trainium2 kernel — v3.

Strategy: shard by cluster ("bucket"). The host groups point indices by
bucket (argsort of cluster_ids — pure index preprocessing), pads each
bucket's point list to a multiple of 16 (a "slot") by duplicating the
bucket's first point, and cuts the bucket list into 8 contiguous,
point-balanced core shards. Every core owns whole buckets: NO cross-core
communication.

v3: the slot->bucket combine avoids bulk ap_gather (measured ~28ns/index
on gpsimd) almost entirely:
  - bucket SUMS: transposed per-slot sums (one-hot PE matmuls with slots
    on partitions) + banded one-hot M matmuls (slot-tile x bucket-chunk).
    Band membership is static: the union over all 8 cores (program is
    compiled per input, SPMD-shared).
  - bucket MAXes: sparse-table (RMQ) — sliding-window max arrays P1,P2 on
    DVE, then ONE ap_gather of just 2 indices per bucket.
  - context expansion (bucket -> slot): banded one-hot E matmuls from
    transposed tables tblT[b,f]; tblT built directly by operand-swapped
    matmuls (lhsT = xmean/vmean/vmax columns). Per-feature biases+scales
    fold into the PSUM->SBUF expansion copies.
All point data is bf16; x is laid out tile-interleaved so DMA partition
lines are 4KB contiguous.

  pass 1: x tiles -> PE transpose -> k^T,v^T matmuls; transposed slot
          x-sums (PE one-hot); per-slot maxes of k^T,v^T (DVE).
  mid:    P1/P2 max tables; 2080-idx gather; banded matmuls -> tables ->
          banded expansion -> ctx3 [128, 3, NSLOT].
  pass 2: x^T -> q^T; interaction = (q^T+bq)*ctx; gate MLP;
          out^T = Wp.T @ (gate*v_comb) + bp.
"""

import numpy as np
import ml_dtypes

import concourse.bass as bass
from concourse import bacc
import concourse.mybir as mybir
import concourse.tile as tile
import concourse.bass_utils as bass_utils
from concourse.masks import make_identity

# problem constants (hardcoded per contract)
N_GLOBAL = 500000
C = 128
B_GLOBAL = 8192
NUM_HEADS = 4
NCORES = 8
SLOT = 16           # points per slot (pad unit)
CHUNK = 512         # points per matmul chunk
QUAD = 2048         # points per DMA load (4 chunks)
TPC = CHUNK // 128  # 128-row tiles per chunk
TPQ = QUAD // 128   # 128-row tiles per quad
SBK = 512           # slots per expansion block
SCALE = float((C // NUM_HEADS) ** (-0.5))
NEG_BIG = -1.0e30

f32 = mybir.dt.float32
bf16 = mybir.dt.bfloat16
i16 = mybir.dt.int16
X = mybir.AxisListType.X
ALU = mybir.AluOpType
ACTF = mybir.ActivationFunctionType
BF16 = ml_dtypes.bfloat16


def _wrap16(vals):
    """ap_gather index layout: [128, n//16] int16, idx j read from
    partition j%16 (replicated across the 8 gpsimd cores' groups)."""
    v = np.asarray(vals, np.int16)
    n = v.size
    assert n % 16 == 0
    arr = np.zeros((128, n // 16), np.int16)
    k = np.arange(n)
    for g in range(8):
        arr[16 * g + (k % 16), k // 16] = v
    return arr


def _build_layout(ids, B, ncores):
    """Bucket-grouped, slot-padded permutation + all per-core metadata."""
    N = ids.shape[0]
    counts = np.bincount(ids, minlength=B).astype(np.int64)
    order = np.argsort(ids, kind="stable")
    starts = np.zeros(B + 1, np.int64)
    starts[1:] = np.cumsum(counts)
    nslots_b = (counts + SLOT - 1) // SLOT
    padded_b = nslots_b * SLOT

    cum = np.cumsum(padded_b)
    total = cum[-1]
    cuts = [0]
    for c in range(1, ncores):
        cuts.append(int(np.searchsorted(cum, c * total / ncores)))
    cuts.append(B)

    core_npts = [int(padded_b[cuts[c]:cuts[c + 1]].sum()) for c in range(ncores)]
    core_nbux = [cuts[c + 1] - cuts[c] for c in range(ncores)]
    NPTS = max(QUAD, int(-(-max(core_npts) // QUAD)) * QUAD)
    NBUX = max(16, int(-(-max(core_nbux) // 16)) * 16)
    NSLOT = NPTS // SLOT
    SPB = max(1, int(nslots_b.max()))  # max slots per bucket
    NTILE = NPTS // 128
    KMAX = max(1, (SPB - 1).bit_length() - 1)  # deepest P table needed
    REG = NSLOT + 16                           # P-region stride in A
    NCHK = -(-NBUX // 128)                     # bucket chunks (last may be short)
    NSB = -(-NSLOT // SBK)                     # slot expansion blocks

    cores = []
    for c in range(ncores):
        bs, be = cuts[c], cuts[c + 1]
        nb = be - bs
        cnts = counts[bs:be]
        nsl = nslots_b[bs:be]
        sstart = np.zeros(nb + 1, np.int64)
        sstart[1:] = np.cumsum(nsl)
        tot_slots = int(sstart[-1])
        dst0 = sstart[:-1] * SLOT

        perm = np.full(NPTS, -1, np.int64)
        is_real = np.zeros(NPTS, bool)
        src = order[starts[bs]:starts[be]]
        if src.size:
            seg0 = (starts[bs:be] - starts[bs])
            pos = np.repeat(dst0, cnts) + (np.arange(src.size) - np.repeat(seg0, cnts))
            perm[pos] = src
            is_real[pos] = True
        padcnt = (nsl * SLOT - cnts)
        if padcnt.sum():
            off = np.concatenate([[0], np.cumsum(padcnt)])[:-1]
            ppos = np.repeat(dst0 + cnts, padcnt) + (np.arange(int(padcnt.sum())) - np.repeat(off, padcnt))
            first = np.where(cnts > 0, order[starts[bs:be]], 0)
            perm[ppos] = np.repeat(first, padcnt)

        si = np.zeros((NPTS, 8), np.float32)
        p_idx = np.arange(NPTS) % 128
        si[np.arange(NPTS), p_idx // 16] = is_real.astype(np.float32)
        # [128, NTILE, 8]: partition-major so the DMA is contiguous
        slot_ind = si.reshape(NTILE, 128, 8).transpose(1, 0, 2).copy()

        cnt_rep = np.ones(NBUX, np.float32)
        cnt_rep[:nb] = np.maximum(cnts, 1).astype(np.float32)
        cnt_rep = np.broadcast_to(cnt_rep, (128, NBUX)).copy()

        # bucket of each slot (NBUX => virtual)
        bos = np.full(NSLOT, NBUX, np.int64)
        if tot_slots:
            bos[:tot_slots] = np.repeat(np.arange(nb), nsl)

        # RMQ gather indices: 2 per bucket into the A = [P0|P1|..|P_KMAX] array
        idx = np.full(2 * NBUX, NSLOT, np.int64)  # default: P0 pad (NEG)
        nz = np.nonzero(nsl > 0)[0]
        if nz.size:
            ns = nsl[nz]
            kk = np.minimum(np.int64(np.floor(np.log2(ns))), KMAX)
            w = 1 << kk
            assert np.all(2 * w >= ns)
            idx[2 * nz] = kk * REG + sstart[:-1][nz]
            idx[2 * nz + 1] = kk * REG + sstart[:-1][nz] + ns - w
        rmqi = _wrap16(idx)

        cores.append(dict(perm=perm, slot_ind=slot_ind, cnt=cnt_rep,
                          rmqi=rmqi, bos=bos, nb=nb))

    # static (union-over-cores) band structure
    band = [set() for _ in range(NCHK)]
    eband = [set() for _ in range(NSB)]
    for core in cores:
        bos = core["bos"]
        real = bos < NBUX
        ch_of_slot = np.where(real, bos // 128, -1)
        for ti in range(NTILE // SLOT * 0 + (NSLOT // 128)):
            cs = ch_of_slot[ti * 128:(ti + 1) * 128]
            for cc in np.unique(cs[cs >= 0]):
                band[int(cc)].add(ti)
        for j in range(NSB):
            cs = ch_of_slot[j * SBK:min((j + 1) * SBK, NSLOT)]
            for cc in np.unique(cs[cs >= 0]):
                eband[j].add(int(cc))
    band = [sorted(s) if s else [0] for s in band]
    eband = [sorted(s) if s else [0] for s in eband]
    M_ORDER = [(c, ti) for c in range(NCHK) for ti in band[c]]
    E_ORDER = [(j, c) for j in range(NSB) for c in eband[j]]

    # per-core one-hot band matrices
    for core in cores:
        bos = core["bos"]
        Mtab = np.zeros((len(M_ORDER), 128, 128), np.float32)
        for i, (c, ti) in enumerate(M_ORDER):
            bl = bos[ti * 128:(ti + 1) * 128] - c * 128
            s_ok = np.nonzero((bl >= 0) & (bl < min(128, NBUX - c * 128)))[0]
            Mtab[i, s_ok, bl[s_ok]] = 1.0
        Etab = np.zeros((len(E_ORDER), 128, SBK), np.float32)
        for i, (j, c) in enumerate(E_ORDER):
            w_j = min(SBK, NSLOT - j * SBK)
            bl = bos[j * SBK:j * SBK + w_j] - c * 128
            s_ok = np.nonzero((bl >= 0) & (bl < min(128, NBUX - c * 128)))[0]
            Etab[i, bl[s_ok], s_ok] = 1.0
        core["Mtab"] = Mtab
        core["Etab"] = Etab

    return dict(NPTS=NPTS, NBUX=NBUX, NSLOT=NSLOT, SPB=SPB, NTILE=NTILE,
                NQUAD=NPTS // QUAD, KMAX=KMAX, REG=REG, NCHK=NCHK, NSB=NSB,
                band=band, eband=eband, M_ORDER=M_ORDER, E_ORDER=E_ORDER,
                cores=cores)


def _build_nc(L, reps=1):
    NPTS, NBUX, NSLOT = L["NPTS"], L["NBUX"], L["NSLOT"]
    NTILE, NQUAD = L["NTILE"], L["NQUAD"]
    KMAX, REG, NCHK, NSB = L["KMAX"], L["REG"], L["NCHK"], L["NSB"]
    M_ORDER, E_ORDER = L["M_ORDER"], L["E_ORDER"]
    band, eband = L["band"], L["eband"]

    nc = bacc.Bacc("TRN2", target_bir_lowering=False)
    xq = nc.dram_tensor("xq", [NQUAD, 128, TPQ, C], bf16, kind="ExternalInput")
    xtp = nc.dram_tensor("xtp", [C, NPTS], bf16, kind="ExternalInput")
    sind_d = nc.dram_tensor("sind", [128, NTILE, 8], bf16, kind="ExternalInput")
    cnt_d = nc.dram_tensor("cnt", [128, NBUX], f32, kind="ExternalInput")
    rmqi_d = nc.dram_tensor("rmqi", [128, (2 * NBUX) // 16], i16, kind="ExternalInput")
    m_d = nc.dram_tensor("mtab", [len(M_ORDER), 128, 128], bf16, kind="ExternalInput")
    e_d = nc.dram_tensor("etab", [len(E_ORDER), 128, SBK], bf16, kind="ExternalInput")
    w_d = {}
    for nm, sh in [("Wq", [C, C]), ("Wk", [C, C]), ("Wv", [C, C]),
                   ("Wg1", [2 * C, C]), ("Wg2", [C, C]), ("Wvc", [2 * C, C]),
                   ("Wp", [C, C])]:
        w_d[nm] = nc.dram_tensor(nm, sh, bf16, kind="ExternalInput")
    b_d = {}
    for nm in ["bq", "bk", "bv", "bg1", "bg2", "bvc", "bp"]:
        b_d[nm] = nc.dram_tensor(nm, [C], f32, kind="ExternalInput")
    ot = nc.dram_tensor("ot", [C, NPTS], bf16, kind="ExternalOutput")

    def _emit(tc):
        with tc.tile_pool(name="consts", bufs=1) as consts, \
             tc.tile_pool(name="tables", bufs=1) as tables:
            ident = consts.tile([128, 128], bf16)
            make_identity(nc, ident[:])
            w = {}
            for nm in ["Wq", "Wk", "Wv", "Wg2", "Wp"]:
                w[nm] = consts.tile([C, C], bf16, name="w_" + nm, tag="w_" + nm)
                nc.sync.dma_start(out=w[nm][:], in_=w_d[nm][:])
            for nm, sl_ in [("Wg1a", 0), ("Wg1b", 1)]:
                w[nm] = consts.tile([C, C], bf16, name="w_" + nm, tag="w_" + nm)
                nc.sync.dma_start(out=w[nm][:],
                                  in_=w_d["Wg1"][sl_ * C:(sl_ + 1) * C, :])
            for nm, sl_ in [("Wvca", 0), ("Wvcb", 1)]:
                w[nm] = consts.tile([C, C], bf16, name="w_" + nm, tag="w_" + nm)
                nc.sync.dma_start(out=w[nm][:], in_=w_d["Wvc"][sl_ * C:(sl_ + 1) * C, :])
            b = {}
            for nm in b_d:
                b[nm] = consts.tile([C, 1], f32, name="b_" + nm, tag="b_" + nm)
                nc.sync.dma_start(out=b[nm][:], in_=b_d[nm][:, None])
            bk_s = consts.tile([C, 1], f32)
            nc.scalar.activation(out=bk_s[:], in_=b["bk"][:], func=ACTF.Identity,
                                 scale=SCALE)
            bv16 = consts.tile([C, 1], bf16)
            nc.scalar.copy(out=bv16[:], in_=b["bv"][:])

            ctx3 = tables.tile([128, 3, NSLOT], bf16)

            # ---------------- pass 1 ----------------
            with tc.tile_pool(name="slots", bufs=1) as slots:
                # A = [P0 | P1 | .. | P_KMAX]; P0 lanes: 0 = k slot-max, 1 = v
                A = slots.tile([128, (KMAX + 1) * REG, 2], bf16)
                for k in range(KMAX + 1):
                    nc.vector.memset(A[:, k * REG + NSLOT:(k + 1) * REG, :], NEG_BIG)
                xslotT = slots.tile([128, NQUAD, 128], bf16)

                with tc.tile_pool(name="p1w", bufs=3) as p1w, \
                     tc.tile_pool(name="p1ps", bufs=2, space="PSUM") as p1ps, \
                     tc.tile_pool(name="p1xs", bufs=2, space="PSUM") as p1xs:
                    sind_sb = p1w.tile([128, NTILE, 8], bf16, bufs=1)
                    nc.sync.dma_start(out=sind_sb[:], in_=sind_d[:])
                    for cj in range(NQUAD):
                        xt = p1w.tile([128, TPQ, 128], bf16, tag="xt")
                        nc.sync.dma_start(out=xt[:], in_=xq[cj])
                        # slot sums [f, slots-of-quad], then PE-transpose them
                        xs_ps = p1xs.tile([128, TPQ, 8], f32, tag="xs")
                        for t in range(TPQ):
                            nc.tensor.matmul(
                                out=xs_ps[:, t, :], lhsT=xt[:, t, :],
                                rhs=sind_sb[:, cj * TPQ + t, :],
                                start=True, stop=True)
                        xs_sb = p1w.tile([128, 128], bf16, tag="xssb")
                        nc.scalar.copy(out=xs_sb[:],
                                       in_=xs_ps[:].rearrange("p t s -> p (t s)"))
                        xsT_ps = p1xs.tile([128, 128], bf16, tag="xsT")
                        nc.tensor.transpose(out=xsT_ps[:], in_=xs_sb[:],
                                            identity=ident[:])
                        nc.scalar.copy(out=xslotT[:, cj, :], in_=xsT_ps[:])
                        # k/v from the host-pretransposed x (no PE transposes)
                        xt2 = p1w.tile([128, QUAD], bf16, tag="xt2")
                        nc.sync.dma_start(out=xt2[:],
                                          in_=xtp[:, cj * QUAD:(cj + 1) * QUAD])
                        for half in range(QUAD // CHUNK):
                            ci = cj * (QUAD // CHUNK) + half
                            kv_ps = p1ps.tile([128, 2, CHUNK], f32, tag="kv")
                            nc.tensor.matmul(
                                out=kv_ps[:, 0, :], lhsT=w["Wk"][:],
                                rhs=xt2[:, half * CHUNK:(half + 1) * CHUNK],
                                start=True, stop=True)
                            nc.tensor.matmul(
                                out=kv_ps[:, 1, :], lhsT=w["Wv"][:],
                                rhs=xt2[:, half * CHUNK:(half + 1) * CHUNK],
                                start=True, stop=True)
                            nc.vector.tensor_reduce(
                                out=A[:, ci * 32:(ci + 1) * 32, :],
                                in_=kv_ps[:].rearrange("p u (s e) -> p s u e",
                                                       e=SLOT),
                                axis=X, op=ALU.max)

                # ---------------- mid phase ----------------
                with tc.tile_pool(name="midw", bufs=1) as midw, \
                     tc.tile_pool(name="mstrm", bufs=3) as mstrm:
                  with tc.tile_pool(name="midps", bufs=1, space="PSUM") as midps, \
                       tc.tile_pool(name="midsm", bufs=2, space="PSUM") as midsm:
                    # sliding-window max tables (overlap-safe: max is idempotent)
                    SP = NSLOT + 8
                    for k in range(1, KMAX + 1):
                        sh = 1 << (k - 1)
                        nc.vector.tensor_tensor(
                            out=A[:, k * REG:k * REG + SP, :],
                            in0=A[:, (k - 1) * REG:(k - 1) * REG + SP, :],
                            in1=A[:, (k - 1) * REG + sh:(k - 1) * REG + SP + sh, :],
                            op=ALU.max)

                    rmqi_sb = midw.tile([128, (2 * NBUX) // 16], i16, tag="rmqi")
                    nc.sync.dma_start(out=rmqi_sb[:], in_=rmqi_d[:])
                    gmx = midw.tile([128, 2 * NBUX, 2], bf16, tag="gmx")
                    nc.gpsimd.ap_gather(
                        out_ap=gmx[:], in_ap=A[:], idxs_ap=rmqi_sb[:],
                        channels=128, num_elems=(KMAX + 1) * REG, d=2,
                        num_idxs=2 * NBUX)
                    gv4 = gmx[:].rearrange("p (b w) l -> p b w l", w=2)
                    kvmax = midw.tile([128, NBUX, 2], bf16, tag="kvmax")
                    nc.vector.tensor_tensor(out=kvmax[:], in0=gv4[:, :, 0, :],
                                            in1=gv4[:, :, 1, :], op=ALU.max)

                    cnt_sb = midw.tile([128, NBUX], f32, tag="cnt")
                    nc.sync.dma_start(out=cnt_sb[:], in_=cnt_d[:])
                    rc = midw.tile([128, NBUX], f32, tag="rc")
                    nc.vector.reciprocal(out=rc[:], in_=cnt_sb[:])

                    # bucket x-sums via banded one-hot matmuls
                    xbs_ps = midps.tile([128, NBUX], f32, tag="big")
                    mi = 0
                    for c in range(NCHK):
                        c0 = c * 128
                        wc = min(128, NBUX - c0)
                        for i, ti in enumerate(band[c]):
                            mt = mstrm.tile([128, 128], bf16, tag="mt")
                            nc.sync.dma_start(out=mt[:], in_=m_d[mi])
                            nc.tensor.matmul(
                                out=xbs_ps[:, c0:c0 + wc], lhsT=xslotT[:, ti, :],
                                rhs=mt[:, :wc], start=(i == 0),
                                stop=(i == len(band[c]) - 1))
                            mi += 1
                    xmean = midw.tile([128, NBUX], bf16, tag="xmean")
                    nc.vector.tensor_tensor(out=xmean[:], in0=xbs_ps[:],
                                            in1=rc[:], op=ALU.mult)

                    # vm_raw = Wv.T @ xmean (no bias; bv folds into bvc_eff)
                    vm_ps = midps.tile([128, NBUX], f32, tag="big")
                    for j in range(0, NBUX, 512):
                        e = min(j + 512, NBUX)
                        nc.tensor.matmul(out=vm_ps[:, j:e], lhsT=w["Wv"][:],
                                         rhs=xmean[:, j:e], start=True, stop=True)
                    vmean = midw.tile([128, NBUX], bf16, tag="vmean")
                    nc.scalar.copy(out=vmean[:], in_=vm_ps[:])

                    # bvc_eff = (Wvca+Wvcb).T @ bv + bvc
                    bvc_ps = midsm.tile([128, 1], f32, tag="sm1", bufs=1)
                    nc.tensor.matmul(out=bvc_ps[:], lhsT=w["Wvca"][:], rhs=bv16[:],
                                     start=True, stop=False)
                    nc.tensor.matmul(out=bvc_ps[:], lhsT=w["Wvcb"][:], rhs=bv16[:],
                                     start=False, stop=True)
                    bvc_eff = midw.tile([128, 1], f32, tag="bvce")
                    nc.scalar.activation(out=bvc_eff[:], in_=bvc_ps[:],
                                         func=ACTF.Identity, bias=b["bvc"][:])

                    # transposed tables tblT[b_loc, (chunk, lane, f)]
                    tblT = midw.tile([128, NCHK, 3, 128], bf16, tag="tblT")
                    for c in range(NCHK):
                        c0 = c * 128
                        wc = min(128, NBUX - c0)
                        kmT_ps = midsm.tile([128, 128], f32, tag="sm")
                        nc.tensor.matmul(out=kmT_ps[:wc, :], lhsT=xmean[:, c0:c0 + wc],
                                         rhs=w["Wk"][:], start=True, stop=True)
                        nc.scalar.copy(out=tblT[:wc, c, 0, :], in_=kmT_ps[:wc, :])
                        kxT_ps = midsm.tile([128, 128], bf16, tag="smT", bufs=1)
                        nc.tensor.transpose(out=kxT_ps[:wc, :],
                                            in_=kvmax[:, c0:c0 + wc, 0],
                                            identity=ident[:])
                        nc.scalar.copy(out=tblT[:wc, c, 1, :], in_=kxT_ps[:wc, :])
                        vcT_ps = midsm.tile([128, 128], f32, tag="sm")
                        nc.tensor.matmul(out=vcT_ps[:wc, :], lhsT=vmean[:, c0:c0 + wc],
                                         rhs=w["Wvca"][:], start=True, stop=False)
                        nc.tensor.matmul(out=vcT_ps[:wc, :],
                                         lhsT=kvmax[:, c0:c0 + wc, 1],
                                         rhs=w["Wvcb"][:], start=False, stop=True)
                        nc.scalar.copy(out=tblT[:wc, c, 2, :], in_=vcT_ps[:wc, :])

                  # banded expansion: ctx3[:, lane, slots] (midps/midsm closed;
                  # expps gets the freed PSUM banks)
                  with tc.tile_pool(name="expps", bufs=2, space="PSUM") as expps:
                    ei = 0
                    for j in range(NSB):
                        s0 = j * SBK
                        wj = min(SBK, NSLOT - s0)
                        ets = []
                        for c in eband[j]:
                            et = mstrm.tile([128, SBK], bf16, tag="et")
                            nc.sync.dma_start(out=et[:], in_=e_d[ei])
                            ets.append((c, et))
                            ei += 1
                        ex_ps = expps.tile([128, 3, SBK], f32, tag="ex")
                        for lane in range(3):
                            for i, (c, et) in enumerate(ets):
                                wc = min(128, NBUX - c * 128)
                                nc.tensor.matmul(
                                    out=ex_ps[:, lane, :wj],
                                    lhsT=tblT[:wc, c, lane, :],
                                    rhs=et[:wc, :wj], start=(i == 0),
                                    stop=(i == len(ets) - 1))
                        nc.scalar.activation(out=ctx3[:, 0, s0:s0 + wj],
                                             in_=ex_ps[:, 0, :wj],
                                             func=ACTF.Identity, scale=SCALE,
                                             bias=bk_s[:])
                        nc.scalar.activation(out=ctx3[:, 1, s0:s0 + wj],
                                             in_=ex_ps[:, 1, :wj],
                                             func=ACTF.Identity, scale=SCALE,
                                             bias=bk_s[:])
                        nc.scalar.activation(out=ctx3[:, 2, s0:s0 + wj],
                                             in_=ex_ps[:, 2, :wj],
                                             func=ACTF.Identity,
                                             bias=bvc_eff[:])

            # ---------------- pass 2 ----------------
            with tc.tile_pool(name="p2w", bufs=3) as p2w, \
                 tc.tile_pool(name="p2ps", bufs=2, space="PSUM") as p2ps, \
                 tc.tile_pool(name="p2po", bufs=2, space="PSUM") as p2po:
                for cj in range(NQUAD):
                    xT2 = p2w.tile([128, QUAD], bf16, tag="xT2")
                    nc.sync.dma_start(out=xT2[:],
                                      in_=xtp[:, cj * QUAD:(cj + 1) * QUAD])
                    oT4 = p2w.tile([128, QUAD // CHUNK, CHUNK], bf16, tag="oTs")
                    for half in range(QUAD // CHUNK):
                        ci = cj * (QUAD // CHUNK) + half
                        sl0 = ci * 32
                        qT_ps = p2ps.tile([128, CHUNK], f32, tag="qT")
                        nc.tensor.matmul(
                            out=qT_ps[:], lhsT=w["Wq"][:],
                            rhs=xT2[:, half * CHUNK:(half + 1) * CHUNK],
                            start=True, stop=True)
                        inter = p2w.tile([128, 2, 32, SLOT], bf16, tag="inter")
                        qv = qT_ps[:].rearrange("p (s e) -> p s e", e=SLOT)
                        for u in range(2):
                            cvu = ctx3[:, u:u + 1, sl0:sl0 + 32].rearrange(
                                "p o s -> p s o")
                            nc.vector.scalar_tensor_tensor(
                                out=inter[:, u], in0=qv, scalar=b["bq"][:],
                                in1=cvu.broadcast_to([128, 32, SLOT]),
                                op0=ALU.add, op1=ALU.mult)
                        h1_ps = p2ps.tile([128, CHUNK], f32, tag="h1")
                        nc.tensor.matmul(
                            out=h1_ps[:], lhsT=w["Wg1a"][:],
                            rhs=inter[:, 0].rearrange("p a b -> p (a b)"),
                            start=True, stop=False)
                        nc.tensor.matmul(
                            out=h1_ps[:], lhsT=w["Wg1b"][:],
                            rhs=inter[:, 1].rearrange("p a b -> p (a b)"),
                            start=False, stop=True)
                        h1 = p2w.tile([128, CHUNK], bf16, tag="h1s")
                        nc.scalar.activation(out=h1[:], in_=h1_ps[:], func=ACTF.Relu,
                                             bias=b["bg1"][:])
                        h2_ps = p2ps.tile([128, CHUNK], f32, tag="h2")
                        nc.tensor.matmul(out=h2_ps[:], lhsT=w["Wg2"][:],
                                         rhs=h1[:], start=True, stop=True)
                        gate = p2w.tile([128, CHUNK], bf16, tag="gate")
                        nc.scalar.activation(out=gate[:], in_=h2_ps[:],
                                             func=ACTF.Sigmoid, bias=b["bg2"][:])
                        gv = p2w.tile([128, 32, SLOT], bf16, tag="gv")
                        cv2 = ctx3[:, 2:3, sl0:sl0 + 32].rearrange("p o s -> p s o")
                        nc.gpsimd.tensor_tensor(
                            out=gv[:],
                            in0=gate[:].rearrange("p (s e) -> p s e", e=SLOT),
                            in1=cv2.broadcast_to([128, 32, SLOT]),
                            op=ALU.mult)
                        oT_ps = p2po.tile([128, CHUNK], f32, tag="oT")
                        nc.tensor.matmul(
                            out=oT_ps[:], lhsT=w["Wp"][:],
                            rhs=gv[:].rearrange("p a b -> p (a b)"),
                            start=True, stop=True)
                        if ci % 2 == 0:
                            nc.vector.tensor_scalar(out=oT4[:, half, :], in0=oT_ps[:],
                                                    scalar1=b["bp"][:], scalar2=None,
                                                    op0=ALU.add)
                        else:
                            nc.scalar.activation(out=oT4[:, half, :], in_=oT_ps[:],
                                                 func=ACTF.Identity, bias=b["bp"][:])
                    nc.gpsimd.dma_start(
                        out=ot[:, cj * QUAD:(cj + 1) * QUAD],
                        in_=oT4[:].rearrange("p a b -> p (a b)"))

    with tile.TileContext(nc) as tc:
        for _rep in range(reps):
            _emit(tc)
    nc.finalize()
    return nc


def _make_in_maps(inputs, layout):
    shared = {}
    for nm in ["Wq", "Wk", "Wv", "Wg1", "Wg2", "Wvc", "Wp"]:
        shared[nm] = np.ascontiguousarray(inputs[nm], BF16)
    for nm in ["bq", "bk", "bv", "bg1", "bg2", "bvc", "bp"]:
        shared[nm] = np.ascontiguousarray(inputs[nm], np.float32)
    x = np.ascontiguousarray(inputs["x"], np.float32)
    NPTS, NQUAD = layout["NPTS"], layout["NQUAD"]
    in_maps = []
    for core in layout["cores"]:
        perm = core["perm"]
        xp = np.zeros((NPTS, C), np.float32)
        m = perm >= 0
        xp[m] = x[perm[m]]
        xp16 = xp.astype(BF16)
        xq = np.ascontiguousarray(
            xp16.reshape(NQUAD, TPQ, 128, C).transpose(0, 2, 1, 3))
        in_maps.append(dict(shared, xq=xq,
                            xtp=np.ascontiguousarray(xp16.T),
                            sind=core["slot_ind"].astype(BF16),
                            cnt=core["cnt"], rmqi=core["rmqi"],
                            mtab=core["Mtab"].astype(BF16),
                            etab=core["Etab"].astype(BF16)))
    return in_maps


def _assemble_out(results, layout, n):
    out = np.empty((n, C), np.float32)
    for core, r in zip(layout["cores"], results):
        perm = core["perm"]
        m = perm >= 0
        out[perm[m]] = np.asarray(r["ot"]).astype(np.float32).T[m]
    return out


def _run(inputs, layout, trace=False):
    nc = _build_nc(layout)
    in_maps = _make_in_maps(inputs, layout)
    res = bass_utils.run_bass_kernel_spmd(
        nc, in_maps, core_ids=list(range(NCORES)), trace=trace)
    out = _assemble_out(res.results, layout, inputs["x"].shape[0])
    return out, res


def kernel(**inputs):
    ids = np.asarray(inputs["cluster_ids"]).astype(np.int64)
    B = int(inputs["total_buckets"])
    layout = _build_layout(ids, B, NCORES)
    out, _ = _run(inputs, layout, trace=False)
    return out


# ---------------------------------------------------------------------------
# pure-numpy emulation of the device program (for logic validation off-HW)
def kernel_emulate(**inputs):
    ids = np.asarray(inputs["cluster_ids"]).astype(np.int64)
    B = int(inputs["total_buckets"])
    L = _build_layout(ids, B, NCORES)
    NPTS, NBUX, NSLOT = L["NPTS"], L["NBUX"], L["NSLOT"]
    KMAX, REG, NCHK, NSB = L["KMAX"], L["REG"], L["NCHK"], L["NSB"]
    x = np.asarray(inputs["x"], np.float32)
    W = {k: np.asarray(inputs[k], np.float32) for k in
         ["Wq", "Wk", "Wv", "Wg1", "Wg2", "Wvc", "Wp",
          "bq", "bk", "bv", "bg1", "bg2", "bvc", "bp"]}
    n = x.shape[0]
    out = np.empty((n, C), np.float32)
    for core in L["cores"]:
        perm = core["perm"]
        m = perm >= 0
        xp = np.zeros((NPTS, C), np.float32)
        xp[m] = x[perm[m]]
        sind = core["slot_ind"]  # [128, NTILE, 8]
        # pass 1
        kT = (xp @ W["Wk"]).T  # pre-bias
        vT = (xp @ W["Wv"]).T
        A = np.full((128, (KMAX + 1) * REG, 2), NEG_BIG, np.float32)
        A[:, :NSLOT, 0] = kT.reshape(128, NSLOT, SLOT).max(axis=2)
        A[:, :NSLOT, 1] = vT.reshape(128, NSLOT, SLOT).max(axis=2)
        SP = NSLOT + 8
        for k in range(1, KMAX + 1):
            sh = 1 << (k - 1)
            A[:, k * REG:k * REG + SP, :] = np.maximum(
                A[:, (k - 1) * REG:(k - 1) * REG + SP, :],
                A[:, (k - 1) * REG + sh:(k - 1) * REG + SP + sh, :])
        # slot sums via indicator (pads zeroed)
        ind = np.zeros((NPTS, NSLOT), np.float32)
        srow = np.arange(NPTS) // SLOT
        pidx = np.arange(NPTS) % 128
        tidx = np.arange(NPTS) // 128
        ind[np.arange(NPTS), srow] = sind[pidx, tidx, pidx // 16]
        xslot = xp.T @ ind  # [128f, NSLOT]
        # mid
        def unwrap(arr, nn_):
            outv = np.zeros(nn_, np.int64)
            k = np.arange(nn_)
            outv[k] = arr[(k % 16), k // 16]
            return outv
        rid = unwrap(core["rmqi"], 2 * NBUX)
        gmx = A[:, rid, :].reshape(128, NBUX, 2, 2)
        kvmax = gmx.max(axis=2)  # [128, NBUX, 2]
        xbs = np.zeros((128, NBUX), np.float32)
        for i, (c, ti) in enumerate(L["M_ORDER"]):
            c0 = c * 128
            wc = min(128, NBUX - c0)
            xbs[:, c0:c0 + wc] += xslot[:, ti * 128:(ti + 1) * 128] @ \
                core["Mtab"][i][:, :wc]
        rc = 1.0 / core["cnt"]
        xmean = xbs * rc
        vm_raw = W["Wv"].T @ xmean
        bvc_eff = (W["Wvc"][:C] + W["Wvc"][C:]).T @ W["bv"] + W["bvc"]
        tblT = np.zeros((128, NCHK, 3, 128), np.float32)
        for c in range(NCHK):
            c0 = c * 128
            wc = min(128, NBUX - c0)
            tblT[:wc, c, 0, :] = xmean[:, c0:c0 + wc].T @ W["Wk"]
            tblT[:wc, c, 1, :] = kvmax[:, c0:c0 + wc, 0].T
            tblT[:wc, c, 2, :] = (vm_raw[:, c0:c0 + wc].T @ W["Wvc"][:C]
                                  + kvmax[:, c0:c0 + wc, 1].T @ W["Wvc"][C:])
        ctx3 = np.zeros((128, 3, NSLOT), np.float32)
        ei = 0
        for j in range(NSB):
            s0 = j * 512
            wj = min(512, NSLOT - s0)
            ex = np.zeros((128, 3, wj), np.float32)
            for c in L["eband"][j]:
                wc = min(128, NBUX - c * 128)
                E = core["Etab"][ei][:wc, :wj]
                for lane in range(3):
                    ex[:, lane, :] += tblT[:wc, c, lane, :].T @ E
                ei += 1
            ctx3[:, 0, s0:s0 + wj] = SCALE * ex[:, 0] + SCALE * W["bk"][:, None]
            ctx3[:, 1, s0:s0 + wj] = SCALE * ex[:, 1] + SCALE * W["bk"][:, None]
            ctx3[:, 2, s0:s0 + wj] = ex[:, 2] + bvc_eff[:, None]
        # pass 2
        qT = (xp @ W["Wq"]).T + W["bq"][:, None]
        ctxe = np.repeat(ctx3, SLOT, axis=2)  # [128, 3, NPTS]
        inter1 = qT * ctxe[:, 0]
        inter2 = qT * ctxe[:, 1]
        h1 = np.maximum(W["Wg1"][:C].T @ inter1 + W["Wg1"][C:].T @ inter2
                        + W["bg1"][:, None], 0.0)
        h2 = W["Wg2"].T @ h1 + W["bg2"][:, None]
        gate = 1.0 / (1.0 + np.exp(-h2))
        gvv = gate * ctxe[:, 2]
        oT = W["Wp"].T @ gvv + W["bp"][:, None]
        out[perm[m]] = oT.T[m]
    return out
